# revision 1
# baseline (speedup 1.0000x reference)
"""Trainium2 Bass kernel for nn_ExperimentNet (SE-style pooling net).

Reference computation (per batch b):
    pool = mean(x[b], axis=(H,W))                # (C,)
    f    = sigmoid(relu(pool @ W1.T) @ W2.T)     # (C,)
    p    = mean(x[b] * f[:,None,None], (H,W))    # (C,)  == f * pool  (f const over H,W)
    out  = p @ W3.T + b3                         # (2,)

Key algebraic identity: mean(x * f) over (H,W) equals f * mean(x), so x is
read exactly ONCE (512 MB total).  Everything after the pooling is a tiny
MLP on (B, C) = (32, 256) values.

Strategy: pure data parallel over 8 NeuronCores, 4 batches per core.
Per core: stream the (4*256, 16384) row-major shard through SBUF, reduce
over the free (spatial) dim on DVE/ACT, then run the whole MLP on-chip
(TensorE matmuls, K split over two 128-partition chunks).  Output (4, 2)
per core, concatenated on host -> (32, 2).

The 1/(H*W) mean scaling is folded into host-prepared W1.T and W3.T copies
(exact: 16384 is a power of two), so the kernel only ever needs raw sums.
"""

import numpy as np

import concourse.bacc as bacc
import concourse.bass as bass
import concourse.mybir as mybir
from concourse import tile
from concourse.bass_utils import run_bass_kernel_spmd

N_CORES = 8
B, C, H, W = 32, 256, 128, 128
S = H * W                  # 16384 spatial elements per (b, c)
B_LOC = B // N_CORES       # 4 batches per core
ROWS = B_LOC * C           # 1024 (b, c) rows per core
P = 128                    # SBUF partitions
G = ROWS // P              # 8 row groups per core
CH = 8192                  # free-dim chunk per DMA (128 x 8192 f32 = 4 MB)
NCH = S // CH              # chunks per row group
CR = C // 4                # 64 hidden units
KC = C // P                # 2 contraction chunks of 128 for C-dim matmuls

FP32 = mybir.dt.float32

_CACHE = {}


def _build_nc():
    nc = bacc.Bacc("TRN2", target_bir_lowering=False, debug=False)

    x_d = nc.dram_tensor("x", [ROWS, S], FP32, kind="ExternalInput")
    w1t_d = nc.dram_tensor("w1t", [C, CR], FP32, kind="ExternalInput")   # W1.T / S
    w2t_d = nc.dram_tensor("w2t", [CR, C], FP32, kind="ExternalInput")   # W2.T
    w3t_d = nc.dram_tensor("w3t", [C, 2], FP32, kind="ExternalInput")    # W3.T / S
    b3b_d = nc.dram_tensor("b3b", [B_LOC, 2], FP32, kind="ExternalInput")
    out_d = nc.dram_tensor("out", [B_LOC, 2], FP32, kind="ExternalOutput")

    with tile.TileContext(nc) as tc:
        with (
            tc.tile_pool(name="xin", bufs=4) as xpool,
            tc.tile_pool(name="small", bufs=1) as spool,
            tc.tile_pool(name="stage", bufs=4) as stpool,
            tc.tile_pool(name="psum", bufs=1, space="PSUM") as ppool,
        ):
            # --- persistent small tiles -------------------------------------
            w1t = []
            w3t = []
            for c in range(KC):
                t1 = spool.tile([P, CR], FP32, tag=f"w1t{c}", name=f"w1t{c}")
                nc.sync.dma_start(t1[:], w1t_d[c * P:(c + 1) * P, :])
                w1t.append(t1)
                t3 = spool.tile([P, 2], FP32, tag=f"w3t{c}", name=f"w3t{c}")
                nc.sync.dma_start(t3[:], w3t_d[c * P:(c + 1) * P, :])
                w3t.append(t3)
            w2t = spool.tile([CR, C], FP32, tag="w2t")
            nc.sync.dma_start(w2t[:], w2t_d[:])
            b3b = spool.tile([B_LOC, 2], FP32, tag="b3b")
            nc.sync.dma_start(b3b[:], b3b_d[:])

            # poolT[c][p, b] = sum over spatial of x[b, c*128+p, :, :]
            poolT = [
                spool.tile([P, B_LOC], FP32, tag=f"poolT{c}", name=f"poolT{c}")
                for c in range(KC)
            ]

            # --- streaming reduction over x ---------------------------------
            for g in range(G):
                b_idx, c_idx = divmod(g, KC)
                stage = stpool.tile([P, NCH], FP32, tag="stage")
                for j in range(NCH):
                    xt = xpool.tile([P, CH], FP32, tag="xt")
                    nc.sync.dma_start(
                        xt[:], x_d[g * P:(g + 1) * P, j * CH:(j + 1) * CH]
                    )
                    nc.vector.reduce_sum(
                        stage[:, j:j + 1], xt[:], axis=mybir.AxisListType.X
                    )
                nc.vector.reduce_sum(
                    poolT[c_idx][:, b_idx:b_idx + 1], stage[:],
                    axis=mybir.AxisListType.X,
                )

            # --- tiny MLP on-chip -------------------------------------------
            # f1T (CR, B_LOC) = (W1/S) @ pool.T ; relu
            ps_f1 = ppool.tile([CR, B_LOC], FP32, tag="ps_f1")
            for c in range(KC):
                nc.tensor.matmul(
                    ps_f1[:], w1t[c][:], poolT[c][:],
                    start=(c == 0), stop=(c == KC - 1),
                )
            f1 = spool.tile([CR, B_LOC], FP32, tag="f1")
            nc.scalar.activation(
                f1[:], ps_f1[:], mybir.ActivationFunctionType.Relu
            )

            # f2T chunk c (P, B_LOC) = W2[c*128:(c+1)*128, :] @ f1T ; sigmoid
            # then p = f2 * pool_sum
            pT = []
            for c in range(KC):
                ps_f2 = ppool.tile([P, B_LOC], FP32, tag=f"ps_f2{c}",
                                   name=f"ps_f2{c}")
                nc.tensor.matmul(
                    ps_f2[:], w2t[:, c * P:(c + 1) * P], f1[:],
                    start=True, stop=True,
                )
                f2 = spool.tile([P, B_LOC], FP32, tag=f"f2{c}", name=f"f2{c}")
                nc.scalar.activation(
                    f2[:], ps_f2[:], mybir.ActivationFunctionType.Sigmoid
                )
                pt = spool.tile([P, B_LOC], FP32, tag=f"pT{c}", name=f"pT{c}")
                nc.vector.tensor_mul(pt[:], f2[:], poolT[c][:])
                pT.append(pt)

            # out (B_LOC, 2) = p @ (W3.T/S) + b3
            ps_o = ppool.tile([B_LOC, 2], FP32, tag="ps_o")
            for c in range(KC):
                nc.tensor.matmul(
                    ps_o[:], pT[c][:], w3t[c][:],
                    start=(c == 0), stop=(c == KC - 1),
                )
            res = spool.tile([B_LOC, 2], FP32, tag="res")
            nc.vector.tensor_add(res[:], ps_o[:], b3b[:])
            nc.sync.dma_start(out_d[:], res[:])

    nc.compile()
    return nc


def _get_nc():
    if "nc" not in _CACHE:
        _CACHE["nc"] = _build_nc()
    return _CACHE["nc"]


def kernel(x, W1, W2, W3, b3, **_unused):
    x = np.ascontiguousarray(np.asarray(x, dtype=np.float32))
    w1t = (np.asarray(W1, np.float32).T / np.float32(S)).astype(np.float32)
    w1t = np.ascontiguousarray(w1t)                       # (C, CR)
    w2t = np.ascontiguousarray(np.asarray(W2, np.float32).T)   # (CR, C)
    w3t = np.ascontiguousarray(
        (np.asarray(W3, np.float32).T / np.float32(S)).astype(np.float32)
    )                                                     # (C, 2)
    b3b = np.ascontiguousarray(
        np.broadcast_to(np.asarray(b3, np.float32)[None, :], (B_LOC, 2))
    )

    nc = _get_nc()
    in_maps = [
        {
            "x": x[i * B_LOC:(i + 1) * B_LOC].reshape(ROWS, S),
            "w1t": w1t,
            "w2t": w2t,
            "w3t": w3t,
            "b3b": b3b,
        }
        for i in range(N_CORES)
    ]
    res = run_bass_kernel_spmd(nc, in_maps, list(range(N_CORES)))
    out = np.concatenate(
        [res.results[i]["out"] for i in range(N_CORES)], axis=0
    )
    return out.astype(np.float32)
